# revision 40
# baseline (speedup 1.0000x reference)
"""Block-causal attention (B=2, S=2048, D=1024, H=16, HD=64, BLOCK=16) on 8 TRN2 cores.

Sharding: core c -> batch c//4, head-group c%4 (4 heads). Each core computes the
full attention for its 4 heads plus a partial out-projection y^T (1024, 2048) in
f16; the host sums the 4 partials per batch (row-parallel unshard) and
transposes.

v2 schedule (single fused phase, everything flows through one 4-bank PSUM ring
plus a 2+2-bank attention set):
  - QK proj for head-pair mt0 runs kt-outer over 4 PSUM banks WHILE x^T streams
    in from HBM; V proj st-outer follows, feeding attention incrementally.
  - RMS sums land on 32-aligned partition pairs of one [98,512] PSUM tile via
    col-tiled ones-matmuls, so the Ln/Exp rrms chain is 512 wide, not 2048.
  - attention processes 512-wide query windows; the pair's two heads occupy PE
    rows 0-63/64-127 (K=64 each) so their score matmuls run concurrently; the
    block-causal mask is a rank-8 matmul (partitions 96-103 for half 0, 0-7 for
    half 1, so mask MMs also pair with the other half's scores).
  - exp needs no row-max because |scores| <= 8 after RMS norm; softmax
    denominator rides as V's appended ones column (M=65).
  - head-pair mt1's projection and the out-projection overlap the ACT-bound exp
    stream; out-proj j-chunks are emitted as soon as both pairs finish query
    window j.
"""

import numpy as np
import ml_dtypes

import concourse.bass as bass
import concourse.tile as tile
from concourse import bacc
from concourse import mybir
from concourse.bass_utils import run_bass_kernel_spmd

BF16 = ml_dtypes.bfloat16
F32 = mybir.dt.float32
F16 = mybir.dt.float16
BF = mybir.dt.bfloat16

B, S, D, H, HD = 2, 2048, 1024, 16, 64
HLOC = 4          # heads per core
NCORES = 8
EPS = 1e-6
SCALE = HD ** -0.5
MASK_C = 8192.0   # masked-pair score offset; exp underflows to 0.0
NST = 4           # 512-wide seq tiles
NKT = 16          # 128-wide key tiles
NDK = 8           # 128-wide model-dim tiles


def _declare_io(nc):
    def din(name, shape, d=BF):
        return nc.dram_tensor(name, shape, d, kind="ExternalInput").ap()

    io = dict(
        xt_d=din("xt", [D, S]),
        wq_d=din("wq", [128, NDK * 256]),
        wk_d=din("wk", [128, NDK * 256]),
        wv_d=din("wv", [128, NDK * 256]),
        wo_d=din("wo", [128, 2 * D]),
        csq_d=din("csq", [128, S]),
        snq_d=din("snq", [128, S]),
        csk_d=din("csk", [128, S]),
        snk_d=din("snk", [128, S]),
        mu_d=din("mu", [8, 128]),
        mv_d=din("mv", [8, 128]),
        ones2_d=din("ones2", [128, 2]),
        b0_d=din("b0", [128, 1], F32),
        yt_d=nc.dram_tensor("yt", [8, 128, S], F16, kind="ExternalOutput").ap(),
    )
    return io


def _emit(tc, io, u=""):
    """Emit the per-core program. Pure SPMD: identical on all 8 cores."""
    from contextlib import ExitStack

    nc = tc.nc
    A = mybir.ActivationFunctionType
    xt_d = io["xt_d"]

    ctx = ExitStack()
    with ctx:
        consts = ctx.enter_context(tc.tile_pool(name="consts" + u, bufs=1))
        persist = ctx.enter_context(tc.tile_pool(name="persist" + u, bufs=1))
        dscratch = ctx.enter_context(
            tc.tile_pool(name="dscratch" + u, bufs=1, space="DRAM")
        )
        xtp = ctx.enter_context(tc.tile_pool(name="xtp" + u, bufs=1))
        # NOTE: pool bufs are PER TAG.  PSUM budget (8 banks):
        #   projp tags pp0/pp1 (proj ring, also V/out-proj/ms) = 2 banks,
        #   spp spA/spB (two-head score tiles, i-parity ping-pong) = 4,
        #   avp av0/av1 = 2.
        projp = ctx.enter_context(tc.tile_pool(name="projp" + u, bufs=1, space="PSUM"))
        spp = ctx.enter_context(tc.tile_pool(name="spp" + u, bufs=1, space="PSUM"))
        avp = ctx.enter_context(tc.tile_pool(name="avp" + u, bufs=1, space="PSUM"))
        # SBUF scratch pools
        qrawp = ctx.enter_context(tc.tile_pool(name="qrawp" + u, bufs=3))
        ropep = ctx.enter_context(tc.tile_pool(name="ropep" + u, bufs=2))
        sqp = ctx.enter_context(tc.tile_pool(name="sqp" + u, bufs=4))
        rrp = ctx.enter_context(tc.tile_pool(name="rrp" + u, bufs=2))
        rrbp = ctx.enter_context(tc.tile_pool(name="rrbp" + u, bufs=3))
        ptp = ctx.enter_context(tc.tile_pool(name="ptp" + u, bufs=4))
        attw = ctx.enter_context(tc.tile_pool(name="attw" + u, bufs=3))
        ystp = ctx.enter_context(tc.tile_pool(name="ystp" + u, bufs=4))

        # ---- input loads ----
        # x^T on sync/scalar (both HW queues fill); weights+tables on gpsimd in
        # consumption order: wq/wk (mt0 proj) first, then rope tables, wv, wo.
        xt_sb = xtp.tile([128, NDK, S], BF)
        wq_sb = consts.tile([128, NDK, 256], BF)
        wk_sb = consts.tile([128, NDK, 256], BF)
        wv_sb = consts.tile([128, NDK, 256], BF)
        wo_sb = consts.tile([128, 2, D], BF)
        nc.gpsimd.dma_start(out=wq_sb, in_=io["wq_d"].rearrange("p (t m) -> p t m", t=NDK))
        nc.gpsimd.dma_start(out=wk_sb, in_=io["wk_d"].rearrange("p (t m) -> p t m", t=NDK))
        for kt in range(NDK):
            eng = nc.sync if kt % 2 == 0 else nc.scalar
            eng.dma_start(
                out=xt_sb[:, kt, :], in_=xt_d[128 * kt : 128 * (kt + 1), :]
            )
        csq_sb = consts.tile([128, S], BF)
        snq_sb = consts.tile([128, S], BF)
        csk_sb = consts.tile([128, S], BF)
        snk_sb = consts.tile([128, S], BF)
        nc.gpsimd.dma_start(out=csq_sb, in_=io["csq_d"])
        nc.gpsimd.dma_start(out=snq_sb, in_=io["snq_d"])
        nc.gpsimd.dma_start(out=csk_sb, in_=io["csk_d"])
        nc.gpsimd.dma_start(out=snk_sb, in_=io["snk_d"])
        nc.gpsimd.dma_start(out=wv_sb, in_=io["wv_d"].rearrange("p (t m) -> p t m", t=NDK))
        nc.gpsimd.dma_start(out=wo_sb, in_=io["wo_d"].rearrange("p (t m) -> p t m", t=2))
        # mask factors at partitions 0-7 (pairs half-1 scores on rows 64-127)
        # and 96-103 (pairs half-0 scores on rows 0-63)
        muv = consts.tile([128, 2, 128], BF)
        for pb in (0, 96):
            nc.sync.dma_start(out=muv[pb : pb + 8, 0, :], in_=io["mu_d"])
            nc.sync.dma_start(out=muv[pb : pb + 8, 1, :], in_=io["mv_d"])
        ones2_sb = consts.tile([128, 2], BF)
        nc.sync.dma_start(out=ones2_sb, in_=io["ones2_d"])
        b0_sb = consts.tile([128, 1], F32)
        nc.sync.dma_start(out=b0_sb, in_=io["b0_d"])
        eps_sb = consts.tile([128, 1], F32)
        nc.vector.memset(eps_sb, EPS)
        # dummy activation up front: pulls the one ACT_TABLE_LOAD (~2.7us)
        # into the x-load window instead of the first RMS-norm's critical path
        warm_sb = consts.tile([128, 1], F32)
        nc.scalar.activation(warm_sb, eps_sb, A.Exp)

        # ---- persistent activations ----
        qT = persist.tile([128, 2, S], BF)      # (2 heads)*64 rows per mt
        kT = persist.tile([128, 2, S], BF)
        vv = persist.tile([128, NKT, HLOC, HD + 1], BF)   # [V | ones]
        at = persist.tile([128, 2, S], BF)      # normalized attn^T
        rr_dram = dscratch.tile([4, 2, NST, 512], BF)  # (pair, head, st, s%512)

        nc.vector.memset(vv[:, :, :, HD : HD + 1], 1.0)

        def qk_mms(qk_i, mt, accs, kt):
            """One kt-slice of the q/k projection matmuls into accs[st]."""
            wsb = wq_sb if qk_i == 0 else wk_sb
            for st in range(NST):
                nc.tensor.matmul(
                    accs[st],
                    lhsT=wsb[:, kt, 128 * mt : 128 * (mt + 1)],
                    rhs=xt_sb[:, kt, 512 * st : 512 * (st + 1)],
                    start=(kt == 0),
                    stop=(kt == NDK - 1),
                )

        def qk_post(qk_i, mt, accs, act_help=False):
            """Evict + RMS-normalize + rope q or k for head pair mt.
            rrms is folded into the output for both sides, so the attention
            exp runs with constant scale/bias."""
            pair = qk_i * 2 + mt
            cstab = csq_sb if qk_i == 0 else csk_sb
            sntab = snq_sb if qk_i == 0 else snk_sb
            dest = qT if qk_i == 0 else kT
            qraw = qrawp.tile([128, S], BF, tag="qraw")
            ms8 = projp.tile([98, 512], F32, tag="pp0")
            sqs = []
            for st in range(NST):
                sl = slice(512 * st, 512 * (st + 1))
                if act_help:
                    # ACT is idle mid-kernel; keep DVE for the rope muls
                    nc.scalar.copy(qraw[:, sl], accs[st])
                    sq = sqp.tile([128, 512], BF, tag="sq")
                    nc.scalar.activation(sq, accs[st], A.Square)
                else:
                    nc.vector.tensor_copy(qraw[:, sl], accs[st])
                    sq = sqp.tile([128, 512], BF, tag="sq")
                    nc.vector.tensor_mul(sq, qraw[:, sl], qraw[:, sl])
                sqs.append(sq)
            # per-st sumsq pairs land on partitions 32*st (col-tiled matmul)
            for st in range(NST):
                nc.tensor.matmul(
                    ms8[32 * st : 32 * st + 2, :],
                    lhsT=ones2_sb,
                    rhs=sqs[st],
                    start=True,
                    stop=True,
                    tile_position=(0, 32 * st),
                )
            # rrms = exp(-0.5*ln(ms/HD + eps)) on the [98,512] stack
            ln8 = rrp.tile([98, 512], F32, tag="ln8")
            rr8 = rrp.tile([98, 512], BF, tag="rr8")
            nc.scalar.activation(ln8, ms8, A.Ln, bias=eps_sb[0:98], scale=1.0 / HD)
            nc.scalar.activation(rr8, ln8, A.Exp, scale=-0.5)
            for st in range(NST):
                nc.sync.dma_start(
                    out=rr_dram[pair, :, st, :],
                    in_=rr8[32 * st : 32 * st + 2, :],
                )
            # rope on raw values (tables carry qn/kn and the sign fold)
            rot = ropep.tile([128, S], BF, tag="rot")
            for ri, (lo, hi) in enumerate(((0, 32), (32, 64), (64, 96), (96, 128))):
                src_lo = lo + 32 if (lo // 32) % 2 == 0 else lo - 32
                eng = nc.sync if ri % 2 == 0 else nc.scalar
                eng.dma_start(out=rot[lo:hi], in_=qraw[src_lo : src_lo + 32])
            t1 = ropep.tile([128, S], BF, tag="t1")
            t2 = ropep.tile([128, S], BF, tag="t2")
            tsum = ropep.tile([128, S], BF, tag="tsum")
            nc.vector.tensor_mul(t1, qraw, cstab)
            nc.vector.tensor_mul(t2, rot, sntab)
            nc.vector.tensor_add(tsum, t1, t2)
            # broadcast rrms across each head's 64 partitions (via DRAM:
            # zero-step partition reads need a DRAM source)
            rrb = rrbp.tile([128, NST, 512], BF, tag="rrb")
            for hh in range(2):
                nc.gpsimd.dma_start(
                    out=rrb[64 * hh : 64 * (hh + 1)],
                    in_=rr_dram[pair, hh : hh + 1].partition_broadcast(64),
                )
            for st in range(NST):
                sl = slice(512 * st, 512 * (st + 1))
                nc.vector.tensor_mul(dest[:, mt, sl], tsum[:, sl], rrb[:, st, :])

        def qk_proj(qk_i, mt, act_help=False):
            """st-outer q/k projection + post, through the pp0/pp1 ring."""
            tiles = [
                projp.tile([128, 512], F32, tag=f"pp{st % 2}", name=f"pp{st}")
                for st in range(NST)
            ]
            for st in range(NST):
                for kt in range(NDK):
                    nc.tensor.matmul(
                        tiles[st],
                        lhsT=(wq_sb if qk_i == 0 else wk_sb)[
                            :, kt, 128 * mt : 128 * (mt + 1)
                        ],
                        rhs=xt_sb[:, kt, 512 * st : 512 * (st + 1)],
                        start=(kt == 0),
                        stop=(kt == NDK - 1),
                    )
            qk_post(qk_i, mt, tiles, act_help=act_help)

        def v_proj(st):
            """V projection for 128-row seq tile st -> vv[:, st]."""
            ps = projp.tile([128, 256], F32, tag=f"pp{st % 2}")
            for kt in range(NDK):
                nc.tensor.matmul(
                    ps,
                    lhsT=xt_sb[:, kt, 128 * st : 128 * (st + 1)],
                    rhs=wv_sb[:, kt, :],
                    start=(kt == 0),
                    stop=(kt == NDK - 1),
                )
            nc.vector.tensor_copy(
                vv[:, st, :, 0:HD], ps.rearrange("p (h d) -> p h d", h=HLOC)
            )

        def attention_window(mt, qq):
            """One 512-wide query window for head pair mt (heads 2mt, 2mt+1).
            Both heads share one [128, 2, 512] score tile (half 0 in PE rows
            0-63, half 1 in rows 64-127, concurrent matmuls) and one exp."""
            glo = 512 * qq
            kmax = 4 * (qq + 1)
            avs = [
                avp.tile([65, 512], F32, tag=f"av{half}", name=f"av{half}")
                for half in range(2)
            ]
            for i in range(kmax):
                q0 = 128 * i
                lo = max(glo, q0)
                loq = lo - glo
                diag = q0 >= glo
                sp = spp.tile([128, 2, 512], F32, tag=f"sp{'AB'[i % 2]}")
                for half in range(2):
                    po = 64 * half
                    nc.tensor.matmul(
                        sp[:, half, loq:512],
                        lhsT=kT[po : po + 64, mt, 128 * i : 128 * (i + 1)],
                        rhs=qT[po : po + 64, mt, lo : glo + 512],
                        start=True,
                        stop=not diag,
                    )
                if diag:
                    for half in range(2):
                        pb = 96 if half == 0 else 0
                        nc.tensor.matmul(
                            sp[:, half, loq : loq + 128],
                            lhsT=muv[pb : pb + 8, 0, :],
                            rhs=muv[pb : pb + 8, 1, :],
                            start=False,
                            stop=True,
                            tile_position=(pb, 0),
                        )
                pt = ptp.tile([128, 2, 512], BF, tag=f"pt{'AB'[i % 2]}")
                nc.scalar.activation(
                    pt[:, :, loq:512],
                    sp[:, :, loq:512],
                    A.Exp,
                    bias=b0_sb,
                    scale=SCALE,
                )
                for half in range(2):
                    h = 2 * mt + half
                    nc.tensor.matmul(
                        avs[half][:, loq:512],
                        lhsT=vv[:, i, h, :],
                        rhs=pt[:, half, loq:512],
                        start=(i == 0),
                        stop=(i == kmax - 1),
                    )
            # normalize: at[head rows] = av[0:64] * (1 / av[64])
            for half in range(2):
                po = 64 * half
                rden = attw.tile([1, 512], F32, tag="rden")
                nc.vector.reciprocal(rden, avs[half][64:65, :])
                rdb = attw.tile([64, 512], F32, tag="rdb")
                nc.gpsimd.partition_broadcast(rdb, rden, channels=64)
                nc.vector.tensor_mul(
                    at[po : po + 64, mt, glo : glo + 512], avs[half][0:64, :], rdb
                )

        def out_proj(j):
            """Partial out-projection columns [512j, 512j+512)."""
            for m in range(8):
                ps = projp.tile([128, 512], F32, tag=f"pp{m % 2}")
                for mtI in range(2):
                    nc.tensor.matmul(
                        ps,
                        lhsT=wo_sb[:, mtI, 128 * m : 128 * (m + 1)],
                        rhs=at[:, mtI, 512 * j : 512 * (j + 1)],
                        start=(mtI == 0),
                        stop=(mtI == 1),
                    )
                yst = ystp.tile([128, 512], F16, tag="yst")
                nc.vector.tensor_copy(yst, ps)
                nc.sync.dma_start(
                    out=io["yt_d"][m, :, 512 * j : 512 * (j + 1)], in_=yst
                )

        # ---- emission: mt0 q/k overlap the x load; V proj streams into the
        # pair-0 attention; mt1 proj and the out-proj overlap the exp stream.
        # mt0: both q and k accumulate kt-outer WHILE x^T streams in, using
        # the (idle until attention) sp/av PSUM slots as extra accumulators.
        pq0 = projp.tile([128, 512], F32, tag="pp0", name="pq0")
        pq1 = projp.tile([128, 512], F32, tag="pp1", name="pq1")
        spA0 = spp.tile([128, 2, 512], F32, tag="spA", name="spA0")
        spB0 = spp.tile([128, 2, 512], F32, tag="spB", name="spB0")
        avq0 = avp.tile([128, 512], F32, tag="av0", name="avq0")
        avq1 = avp.tile([128, 512], F32, tag="av1", name="avq1")
        q_accs = [pq0, pq1, spB0[:, 0, :], spB0[:, 1, :]]
        k_accs = [spA0[:, 0, :], spA0[:, 1, :], avq0, avq1]
        for kt in range(NDK):
            qk_mms(0, 0, q_accs, kt)
            qk_mms(1, 0, k_accs, kt)
        qk_post(0, 0, q_accs)
        qk_post(1, 0, k_accs)
        qk_proj(0, 1)
        qk_proj(1, 1)
        for st in range(NKT):
            v_proj(st)
        for qq in range(NST):
            attention_window(0, qq)
        for qq in range(NST):
            attention_window(1, qq)
            out_proj(qq)


class _pin_act_table:
    """Force every activation we use (Exp, Ln, Copy) onto the one table set
    containing them all, so the program does a single ACT_TABLE_LOAD."""

    def __init__(self, arch):
        from concourse.hw_specs import get_activation_tables

        self.tabs = get_activation_tables(arch)

    def __enter__(self):
        self.saved = {nm: set(s) for nm, s in self.tabs.items()}
        for nm, s in self.tabs.items():
            if nm != "natural_log_exp_and_others":
                s.clear()

    def __exit__(self, *a):
        for nm, s in self.tabs.items():
            s.clear()
            s.update(self.saved[nm])


def build_program(iters=1):
    nc = bacc.Bacc(
        "TRN2",
        target_bir_lowering=False,
        debug=False,
        enable_asserts=False,
        num_devices=NCORES,
    )
    with tile.TileContext(nc) as tc:
        io = _declare_io(nc)
        for it in range(iters):
            _emit(tc, io, u=f"_i{it}" if iters > 1 else "")
    with _pin_act_table(nc.m.arch):
        nc.compile()
    return nc


def make_core_inputs(x, qkv_w, out_w, qn_w, kn_w, rope_cos, rope_sin, attention_mask):
    """Host-side shard/layout prep. Returns list of 8 per-core input dicts."""
    x = np.asarray(x, np.float32)
    qkv_w = np.asarray(qkv_w, np.float32)
    out_w = np.asarray(out_w, np.float32)
    qn_w = np.asarray(qn_w, np.float32)
    kn_w = np.asarray(kn_w, np.float32)
    rope_cos = np.asarray(rope_cos, np.float32)
    rope_sin = np.asarray(rope_sin, np.float32)
    am = np.asarray(attention_mask)

    r = qkv_w.reshape(3, H, HD, D)
    csT = rope_cos.T.astype(np.float32)                # (64, S)
    snT = rope_sin.T.astype(np.float32)
    s2 = np.concatenate([-snT[0:32], snT[32:64]], axis=0)  # sign-folded sin
    perm = np.concatenate([np.arange(32, 64), np.arange(0, 32)])

    def fold(tab, w, permute):
        ww = w[perm] if permute else w
        t = tab * ww[:, None]
        return np.concatenate([t, t], axis=0).astype(BF16)  # (128, S)

    csq = fold(csT, qn_w, False)
    snq = fold(s2, qn_w, True)
    csk = fold(csT, kn_w, False)
    snk = fold(s2, kn_w, True)

    # rank-8 factorization of the (128,128) diagonal-block mask
    dis = ~(am[0:128, 0:128].T)                        # dis[k', q'] disallowed
    mu = np.zeros((8, 128), np.float32)
    mv = np.zeros((8, 128), np.float32)
    for t in range(8):
        mu[t] = np.arange(128) // 16 == t
        mv[t] = -MASK_C * dis[16 * t, :]
    ones2 = np.zeros((128, 2), np.float32)
    ones2[0:64, 0] = 1.0
    ones2[64:128, 1] = 1.0
    b0 = float(HD * SCALE * max(1e-30, np.abs(qn_w).max() * np.abs(kn_w).max()))
    b0_t = np.full((128, 1), -b0, np.float32)

    shared = dict(
        csq=csq,
        snq=snq,
        csk=csk,
        snk=snk,
        mu=mu.astype(BF16),
        mv=mv.astype(BF16),
        ones2=ones2.astype(BF16),
        b0=b0_t,
    )
    in_maps = []
    for c in range(NCORES):
        b, g = divmod(c, 4)
        hs = slice(HLOC * g, HLOC * (g + 1))
        m = dict(shared)
        m["xt"] = np.ascontiguousarray(x[b].T).astype(BF16)

        def _wlayout(w):
            # (D, M) -> (128, NDK*M): partition p holds [t, m] = w[t*128+p, m]
            mm = w.shape[1]
            return np.ascontiguousarray(
                w.reshape(-1, 128, mm).transpose(1, 0, 2).reshape(128, -1)
            ).astype(BF16)

        m["wq"] = _wlayout(r[0, hs].transpose(2, 0, 1).reshape(D, 256))
        m["wk"] = _wlayout(r[1, hs].transpose(2, 0, 1).reshape(D, 256))
        m["wv"] = _wlayout(r[2, hs].transpose(2, 0, 1).reshape(D, 256))
        m["wo"] = _wlayout(
            np.ascontiguousarray(out_w[:, 256 * g : 256 * (g + 1)].T)
        )
        in_maps.append(m)
    return in_maps


_PROGRAM = []


def get_program():
    if not _PROGRAM:
        _PROGRAM.append(build_program())
    return _PROGRAM[0]


def unshard(results):
    """results: list of 8 dicts with 'yt' (8, 128, 2048) f16 partials."""
    ys = []
    for b in range(B):
        acc = np.zeros((8, 128, S), np.float32)
        for g in range(4):
            acc += np.asarray(results[4 * b + g]["yt"], np.float32)
        yt = acc.reshape(D, S)
        ys.append(yt.T.astype(np.float32))
    return np.stack(ys)


def kernel(**inputs):
    in_maps = make_core_inputs(**inputs)
    nc = get_program()
    res = run_bass_kernel_spmd(nc, in_maps, core_ids=list(range(NCORES)))
    return unshard(res.results)
